# revision 38
# baseline (speedup 1.0000x reference)
"""Trainium2 Bass kernel for nn_CAModel (sobel-conv + 2-layer MLP + masked residual).

Math per pixel: y = [x, sobel_x(x), sobel_y(x)] (48 ch); h = relu(w0 @ y + b0);
u = w1 @ h; out = x + u * (rand_u > 0.5).

Sharding: data-parallel over 8 cores: (batch b, H-half) -> core b*2 + half.
Each core computes a [16, 256, 512] output slab.

Per-core design notes (cost-model-driven rewrite of the v1 kernel):
 - All inputs host-packed bf16; HWDGE DMA count minimized (each DMA holds the
   serial HWDGE ring ~630ns regardless of size).
 - Partition layout p = g*16 + c (groups of GR=16 rows x 16 channels), two
   half-images of 128 rows; 16 row-chunks of 16 rows, software-pipelined.
 - Sobel separable passes: bv (B = Dy x) on Pool, the rest on DVE (Pool
   cannot touch PSUM on HW, so DVE time is precious: T-variant avoids the
   un-2x'd scalar_tensor_tensor).  Sobel for half-image 1 and for the NEXT
   rep's half-image 0 are emitted inside pop-windows (chunks 2-6 / 9-13) so
   DVE computes them while PE crunches other chunks; reps overlap without
   any loop-carried emission (unroll>1 bodies inside one For_i iteration).
 - mm1 K=48 in row pairs -> 2-bank PSUM; one relu+bias evac per 2 rows
   (ACT mostly, DVE where free); mm2 col-tiled (tile_position) into a
   32-stacked PSUM tile.
 - Mask: host sends a replicated stacked mask slab (bf16); u*mask is one DVE
   tensor_tensor in non-window chunks, or ACT copy + Pool SBUF-multiply in
   sobel windows.
 - Residual: SWDGE accumulate-DMA adds the host-stacked x slab from HBM into
   the output staging tile; one batched bf16 store per 16-row chunk.
"""
import numpy as np
from contextlib import ExitStack

import concourse.bass as bass
import concourse.bacc as bacc
import concourse.tile as tile
from concourse import mybir

bf16 = mybir.dt.bfloat16
f32 = mybir.dt.float32
Alu = mybir.AluOpType
Act = mybir.ActivationFunctionType

C = 16          # channels
HID = 128
N_CORES = 8
NPFX = np.dtype(mybir.dt.np(bf16))  # ml_dtypes bfloat16


def build_nc(R=256, WP=514, GR=16, reps=1, unroll=1, ablate=(),
             win_dve_evac=2, psum_swap=False, nw_dve=6):
    """Per-core graph. R: out rows (2 half-images), WP: padded width, GR: rows/group."""
    W = WP - 2
    NH = 2                       # half-images
    RH = R // NH                 # rows per half
    n_g = RH // GR               # groups per half
    assert n_g * GR == RH and n_g * C <= 128
    GRH = GR + 2                 # rows incl halo
    UTG = GR // 4                # u-tiles per group (chunk)
    NP = n_g * C                 # active partitions in group layout
    WH = WP // 2 + 1             # sobel half width (overlap 2)
    n_chunk = R // GR            # output chunks (= total groups)
    SMALL = n_g < 6

    nc = bacc.Bacc()
    xg_ext = [nc.declare_dram_parameter(f"xg{h}", (NP, GRH, WP), bf16,
                                        isOutput=False) for h in range(NH)]
    xs_ext = nc.declare_dram_parameter("xs", (n_chunk, 128, UTG, W), bf16,
                                       isOutput=False)
    ms_ext = nc.declare_dram_parameter("ms", (n_chunk, 128, UTG, W), bf16,
                                       isOutput=False)
    w0_ext = nc.declare_dram_parameter("w0t", (3 * C, HID), bf16, isOutput=False)
    b0_ext = nc.declare_dram_parameter("b0", (HID, 1), f32, isOutput=False)
    w1_ext = nc.declare_dram_parameter("w1t", (HID, 32), bf16, isOutput=False)
    out_ext = nc.declare_dram_parameter("out", (n_chunk, 128, UTG, W), bf16,
                                        isOutput=True)

    with tile.TileContext(nc) as tc, ExitStack() as ctx:
        const = ctx.enter_context(tc.tile_pool(name="const", bufs=1))
        xpool = ctx.enter_context(tc.tile_pool(name="xpool", bufs=1))
        gpool = ctx.enter_context(tc.tile_pool(name="gpool", bufs=1))
        abpool = ctx.enter_context(tc.tile_pool(name="abpool", bufs=1))
        ypool = ctx.enter_context(tc.tile_pool(name="ypool", bufs=3))
        hpool = ctx.enter_context(tc.tile_pool(name="hpool", bufs=3))
        opool = ctx.enter_context(tc.tile_pool(name="opool", bufs=3))
        mpool = ctx.enter_context(tc.tile_pool(name="mpool", bufs=2))
        hpsum = ctx.enter_context(tc.tile_pool(
            name="hpsum", bufs=2 if psum_swap else 3, space="PSUM"))
        upsum = ctx.enter_context(tc.tile_pool(
            name="upsum", bufs=4 if psum_swap else 2, space="PSUM"))

        # ---- registries + prologue constants ----
        Xbs, xvs = [None, None], [None, None]
        gviews = [None, None]

        def emit_xload(h):
            Xbs[h] = xpool.tile([NP, GRH * WP], bf16, tag=f"xb{h}", name=f"xb{h}")
            xvs[h] = Xbs[h][:, :].rearrange("p (r w) -> p r w", r=GRH)
            nc.sync.dma_start(xvs[h][:, :, 0:WH], xg_ext[h][:, :, 0:WH])
            nc.sync.dma_start(xvs[h][:, :, WH:WP], xg_ext[h][:, :, WH:WP])

        W0T = const.tile([3 * C, HID], bf16, tag="w0t")
        nc.sync.dma_start(W0T[:], w0_ext[:])
        W1T = const.tile([HID, 32], bf16, tag="w1t")
        nc.sync.dma_start(W1T[:], w1_ext[:])
        B0 = const.tile([HID, 1], f32, tag="b0")
        nc.sync.dma_start(B0[:], b0_ext[:])

        def make_sobel(h):
            """Build sobel closure lists (pool_ops: bv on Pool; dve_ops: rest)."""
            GX = gpool.tile([NP, GR * W], bf16, tag=f"gx{h}", name=f"gx{h}")
            GY = gpool.tile([NP, GR * W], bf16, tag=f"gy{h}", name=f"gy{h}")
            gxv = GX[:, :].rearrange("p (r w) -> p r w", r=GR)
            gyv = GY[:, :].rearrange("p (r w) -> p r w", r=GR)
            gviews[h] = (gxv, gyv)
            xv = xvs[h]
            pool_ops, dve_ops = [], []
            for h2 in range(2):
                w0_ = h2 * (WP // 2 - 1)
                wc = h2 * (WP // 2 - 1)
                wv = slice(w0_, w0_ + WH)
                WO = WH - 2
                A = abpool.tile([NP, GR * WH], bf16, tag="a", name="a")
                B = abpool.tile([NP, GR * WH], bf16, tag="b", name="b")
                T = abpool.tile([NP, GR * WH], bf16, tag="t", name="t")
                av = A[:, :].rearrange("p (r w) -> p r w", r=GR)
                bv = B[:, :].rearrange("p (r w) -> p r w", r=GR)
                tv = T[:, :].rearrange("p (r w) -> p r w", r=GR)
                # A = x0 + x2 ; T = 2*x1 ; A += T ; GX = A>>1 - A<<1   (DVE)
                # B = x2 - x0 (Pool) ; GY = B<<1 + B>>1 ; T = 2*B ; GY += T
                pool_ops.append(
                    lambda bv=bv, xv=xv, wv=wv: nc.gpsimd.tensor_tensor(
                        bv[:, :, :], xv[:, 2:GR + 2, wv], xv[:, 0:GR, wv],
                        Alu.subtract))
                dve_ops += [
                    lambda av=av, xv=xv, wv=wv: nc.vector.tensor_tensor(
                        av[:, :, :], xv[:, 0:GR, wv], xv[:, 2:GR + 2, wv],
                        Alu.add),
                    lambda tv=tv, xv=xv, wv=wv: nc.vector.tensor_scalar(
                        tv[:, :, :], xv[:, 1:GR + 1, wv], 2.0, None, Alu.mult),
                    lambda av=av, tv=tv: nc.vector.tensor_tensor(
                        av[:, :, :], av[:, :, :], tv[:, :, :], Alu.add),
                    lambda gxv=gxv, av=av, wc=wc, WO=WO: nc.vector.tensor_tensor(
                        gxv[:, :, wc:wc + WO], av[:, :, 2:WH],
                        av[:, :, 0:WH - 2], Alu.subtract),
                    lambda gyv=gyv, bv=bv, wc=wc, WO=WO: nc.vector.tensor_tensor(
                        gyv[:, :, wc:wc + WO], bv[:, :, 0:WH - 2],
                        bv[:, :, 2:WH], Alu.add),
                    lambda tv=tv, bv=bv: nc.vector.tensor_scalar(
                        tv[:, :, :], bv[:, :, :], 2.0, None, Alu.mult),
                    lambda gyv=gyv, tv=tv, wc=wc, WO=WO: nc.vector.tensor_tensor(
                        gyv[:, :, wc:wc + WO], gyv[:, :, wc:wc + WO],
                        tv[:, :, 1:WH - 1], Alu.add),
                ]
            return pool_ops, dve_ops

        def emit_sobel_block(h):
            if 'sobel' in ablate:
                return
            pool_ops, dve_ops = make_sobel(h)
            # interleave so the pool op of each half runs early
            for op in (pool_ops[:1] + dve_ops[:7] + pool_ops[1:] + dve_ops[7:]):
                op()

        # sobel-busy chunks (UM routed ACT-copy + Pool-mult; evacs all-ACT)
        W1 = set(range(2, 7)) if not SMALL else set()
        W0_t = set(range(9, 14)) if not SMALL else set()

        def _body(first=True, last=True):
            sob_q = []
            windows = W1 | (set() if last else W0_t)
            if first:
                emit_xload(0)
                emit_sobel_block(0)
            Ys = [None] * n_chunk
            MSs = [None] * n_chunk
            OFs = [None] * n_chunk

            def pre(c):
                hh = c // n_g
                g = c % n_g
                if hh > 0 and sob_q and c < n_g + 2:
                    for op in sob_q:
                        op()
                    sob_q.clear()
                MSs[c] = mpool.tile([128, UTG * W], bf16, tag="ms", name="ms")
                nc.sync.dma_start(
                    MSs[c][:, :].rearrange("p (j w) -> p j w", j=UTG), ms_ext[c])
                Ys[c] = ypool.tile([3 * C, GR * W], bf16, tag="y", name="y")
                yv = Ys[c][:, :].rearrange("s (r w) -> s r w", r=GR)
                if 'pack' not in ablate:
                    gxv, gyv = gviews[hh]
                    nc.sync.dma_start(
                        yv[0:C, :, :],
                        xvs[hh][g * C:(g + 1) * C, 1:1 + GR, 1:WP - 1])
                    nc.sync.dma_start(yv[C:2 * C, :, :],
                                      gxv[g * C:(g + 1) * C, :, :])
                    nc.sync.dma_start(yv[2 * C:3 * C, :, :],
                                      gyv[g * C:(g + 1) * C, :, :])

            def emit_out(c, eng):
                if 'out' not in ablate:
                    ofv = OFs[c][:, :].rearrange("p (j w) -> p j w", j=UTG)
                    eng.dma_start(out_ext[c], ofv[:, :, :])

            if NH > 1:
                emit_xload(1)
                if SMALL and 'sobel' not in ablate:
                    emit_sobel_block(1)
            pre(0)
            for c in range(n_chunk):
                if c == 1 and NH > 1 and not SMALL and 'sobel' not in ablate:
                    pool_ops, dve_ops = make_sobel(1)
                    for op in pool_ops:
                        op()
                    sob_q.extend(dve_ops)
                if c == 7 and not last and not SMALL:
                    emit_xload(0)
                if c == 8 and not last and not SMALL and 'sobel' not in ablate:
                    pool_ops, dve_ops = make_sobel(0)
                    for op in pool_ops:
                        op()
                    sob_q.extend(dve_ops)
                if c == 0 and n_chunk > 1:
                    pre(1)
                yv = Ys[c][:, :].rearrange("s (r w) -> s r w", r=GR)
                msv = MSs[c][:, :].rearrange("p (j w) -> p j w", j=UTG)
                OFs[c] = opool.tile([128, UTG * W], bf16, tag="of", name="of")
                ofv = OFs[c][:, :].rearrange("p (j w) -> p j w", j=UTG)
                in_win = c in windows
                for j in range(UTG):
                    hsb2 = []
                    for half in range(2):
                        h_ps = hpsum.tile([HID, 2 * W], f32, tag="h2", name="h2")
                        if 'mm1' not in ablate:
                            for kk in range(2):
                                r = 4 * j + 2 * half + kk
                                nc.tensor.matmul(h_ps[:, kk * W:(kk + 1) * W],
                                                 W0T[:], yv[:, r, :],
                                                 start=True, stop=True)
                        h_sb = hpool.tile([HID, 2 * W], bf16, tag="h2s",
                                          name="h2s")
                        hsb2.append(h_sb)
                        if 'evac' not in ablate:
                            u_i = 2 * j + half
                            dve_evac = (u_i % 2 == 0 or
                                        (nw_dve >= 5 and u_i == 1) or
                                        (nw_dve >= 6 and u_i == 3)) \
                                if not in_win \
                                else (win_dve_evac == 4 and u_i % 2 == 1
                                      or win_dve_evac == 2 and u_i % 4 == 1)
                            if dve_evac:
                                nc.vector.tensor_scalar(h_sb[:], h_ps[:],
                                                        B0[:], 0.0,
                                                        Alu.add, Alu.max)
                            else:
                                nc.scalar.activation(h_sb[:], h_ps[:],
                                                     Act.Relu, bias=B0[:])

                    u_ps = upsum.tile([128, W], f32, tag="u", name="u")
                    if 'mm2' not in ablate:
                        for k in range(4):
                            nc.tensor.matmul(
                                u_ps[32 * k:32 * k + 32, :], W1T[:],
                                hsb2[k // 2][:, (k % 2) * W:(k % 2 + 1) * W],
                                start=True, stop=True,
                                tile_position=(0, 32 * k))
                    if 'um' not in ablate:
                        if in_win:
                            nc.scalar.activation(ofv[:, j, :], u_ps[:],
                                                 Act.Copy)
                            nc.gpsimd.tensor_tensor(ofv[:, j, :],
                                                    ofv[:, j, :],
                                                    msv[:, j, :], Alu.mult)
                        else:
                            nc.vector.tensor_tensor(ofv[:, j, :], u_ps[:],
                                                    msv[:, j, :], Alu.mult)
                    # staggered non-compute work
                    if j == 0 and c >= 2:
                        emit_out(c - 2, nc.sync)
                    if j == 1 and c + 2 < n_chunk:
                        pre(c + 2)
                    if sob_q and (in_win or c in (7, 8)):
                        take = 1 if c < n_chunk - 1 else len(sob_q)
                        for op in sob_q[:take]:
                            op()
                        del sob_q[:take]
                if 'accum' not in ablate:
                    nc.gpsimd.dma_start(ofv[:, :, :], xs_ext[c],
                                        accum_op=Alu.add)
            for op in sob_q:
                op()
            for c in range(max(0, n_chunk - 2), n_chunk):
                emit_out(c, nc.scalar)

        assert reps % unroll == 0
        if reps > 1:
            with tc.For_i(0, reps // unroll, 1):
                for r in range(unroll):
                    _body(first=(r == 0), last=(r == unroll - 1))
        else:
            _body()
    return nc


_CACHE = {}


def _get_nc():
    if "nc" not in _CACHE:
        nc = build_nc()
        nc.finalize()
        _CACHE["nc"] = nc
    return _CACHE["nc"]


def _pack_core(xp_b, m_b, R, WP, GR):
    """Per-core input pack. xp_b: [C, R+2, WP] padded bf16 slab (rows incl
    halo), m_b: [R, W] bf16 thresholded mask. Returns dict of arrays."""
    W = WP - 2
    NH = 2
    RH = R // NH
    n_g = RH // GR
    NP = n_g * C
    GRH = GR + 2
    UTG = GR // 4
    n_chunk = R // GR

    d = {}
    for h in range(NH):
        xg = np.empty((NP, GRH, WP), NPFX)
        for g in range(n_g):
            r0 = h * RH + g * GR
            xg[g * C:(g + 1) * C] = xp_b[:, r0:r0 + GRH, :]
        d[f"xg{h}"] = xg
    # xs[chunk, 32k+c, j, w] = x[c, GR*chunk + 4j + k, w] (interior), c>=16 -> 0
    xs = np.zeros((n_chunk, 128, UTG, W), NPFX)
    xint = xp_b[:, 1:R + 1, 1:WP - 1]                  # [C, R, W]
    for k in range(4):
        xs[:, 32 * k:32 * k + C] = (
            xint[:, k::4, :].reshape(C, n_chunk, UTG, W).transpose(1, 0, 2, 3))
    d["xs"] = xs
    # ms[chunk, 32k+c, j, w] = m[GR*chunk + 4j + k, w]  (all c)
    ms = np.empty((n_chunk, 128, UTG, W), NPFX)
    mm = m_b.reshape(n_chunk, UTG, 4, W)
    for k in range(4):
        ms[:, 32 * k:32 * k + 32] = mm[:, :, k][:, None, :, :]
    d["ms"] = ms
    return d


def _shard_inputs(x, w0, b0, w1, rand_u, R=256, WP=514, GR=16):
    B, _, H, Wf = x.shape
    xp = np.pad(x, ((0, 0), (0, 0), (1, 1), (1, 1))).astype(NPFX)
    m = (rand_u[:, 0] > 0.5).astype(NPFX)              # [B, H, W]
    w0t = np.ascontiguousarray(w0.T).astype(NPFX)      # [48, 128]
    b0c = np.ascontiguousarray(b0, np.float32).reshape(HID, 1)
    w1t = np.zeros((HID, 32), NPFX)
    w1t[:, :C] = w1.T
    in_maps = []
    for core in range(N_CORES):
        b, hh = divmod(core, 2)
        rowbase = hh * R
        xp_b = xp[b, :, rowbase:rowbase + R + 2, :]    # [C, R+2, WP]
        m_b = m[b, rowbase:rowbase + R, :]             # [R, W]
        dd = _pack_core(xp_b, m_b, R, WP, GR)
        dd.update({"w0t": w0t, "b0": b0c, "w1t": w1t})
        in_maps.append(dd)
    return in_maps


def _assemble(results, B, H, Wf, R=256, GR=16):
    out = np.empty((B, C, H, Wf), np.float32)
    n_chunk = R // GR
    UTG = GR // 4
    for core, res in enumerate(results):
        b, hh = divmod(core, 2)
        o = np.asarray(res["out"]).astype(np.float32)  # [n_chunk, 128, UTG, W]
        o = o.reshape(n_chunk, 4, 32, UTG, Wf)[:, :, :C]   # [nc, 4(k), 16, UTG, W]
        o = o.transpose(2, 0, 3, 1, 4).reshape(C, R, Wf)
        out[b, :, hh * R:(hh + 1) * R, :] = o
    return out


def kernel(x, w0, b0, w1, rand_u, _trace=False):
    from concourse.bass_utils import run_bass_kernel_spmd
    nc = _get_nc()
    in_maps = _shard_inputs(x, w0, b0, w1, rand_u)
    res = run_bass_kernel_spmd(nc, in_maps, core_ids=list(range(N_CORES)))
    out = _assemble(res.results, x.shape[0], x.shape[2], x.shape[3])
    if _trace:
        return out, res
    return out


def _run_timed(nc, in_maps, iters):
    import time
    import jax
    from concourse import mybir
    from jax.sharding import Mesh, PartitionSpec
    from jax.experimental.shard_map import shard_map
    from concourse import bass2jax
    from concourse.bass2jax import _bass_exec_p

    bass2jax.install_neuronx_cc_hook()

    pname = nc.partition_id_tensor.name if nc.partition_id_tensor else None
    in_names, out_names, out_avals, zero_outs = [], [], [], []
    for alloc in nc.m.functions[0].allocations:
        if not isinstance(alloc, mybir.MemoryLocationSet):
            continue
        name = alloc.memorylocations[0].name
        if alloc.kind == "ExternalInput":
            if name != pname:
                in_names.append(name)
        elif alloc.kind == "ExternalOutput":
            out_names.append(name)
            shape = tuple(alloc.tensor_shape)
            np_dt = mybir.dt.np(alloc.dtype)
            out_avals.append(jax.core.ShapedArray(shape, np_dt))
            zero_outs.append(np.zeros(shape, np_dt))
    n_params = len(in_names)
    all_in = in_names + out_names
    if pname is not None:
        all_in = all_in + [pname]

    def _body(*args):
        operands = list(args)
        if pname is not None:
            operands.append(bass2jax.partition_id_tensor())
        outs = _bass_exec_p.bind(
            *operands, out_avals=tuple(out_avals), in_names=tuple(all_in),
            out_names=tuple(out_names), lowering_input_output_aliases=(),
            sim_require_finite=True, sim_require_nnan=True, nc=nc)
        return tuple(outs)

    devices = jax.devices()[:N_CORES]
    mesh = Mesh(np.asarray(devices), ("core",))
    specs = (PartitionSpec("core"),)
    fn = jax.jit(shard_map(_body, mesh=mesh,
                           in_specs=specs * (n_params + len(out_names)),
                           out_specs=specs * len(out_names), check_rep=False),
                 keep_unused=True)
    concat_in = [np.concatenate([np.asarray(in_maps[c][n]) for c in range(N_CORES)], axis=0)
                 for n in in_names]
    concat_zeros = [np.zeros((N_CORES * z.shape[0], *z.shape[1:]), z.dtype)
                    for z in zero_outs]
    dev_in = [jax.device_put(a) for a in concat_in + concat_zeros]

    outs = fn(*dev_in)
    jax.block_until_ready(outs)
    best = float("inf")
    for _ in range(iters):
        t0 = time.perf_counter()
        outs = fn(*dev_in)
        jax.block_until_ready(outs)
        best = min(best, time.perf_counter() - t0)

    res = [{n: np.asarray(outs[i]).reshape(N_CORES, *out_avals[i].shape)[c]
            for i, n in enumerate(out_names)} for c in range(N_CORES)]
    return res, best


_REPS = 256
_UNROLL = 4


def kernel_timed(x, w0, b0, w1, rand_u, iters=8):
    """Returns (out, est_exec_seconds): marginal per-iteration silicon time
    measured as (wall(reps=N) - wall(reps=1)) / (N-1) on device-resident inputs."""
    in_maps = _shard_inputs(x, w0, b0, w1, rand_u)
    nc1 = _get_nc()
    res, t1 = _run_timed(nc1, in_maps, iters)
    out = _assemble(res, x.shape[0], x.shape[2], x.shape[3])
    if "ncR" not in _CACHE:
        ncR = build_nc(reps=_REPS, unroll=_UNROLL)
        ncR.finalize()
        _CACHE["ncR"] = ncR
    resR, tR = _run_timed(_CACHE["ncR"], in_maps, iters)
    outR = _assemble(resR, x.shape[0], x.shape[2], x.shape[3])
    assert np.array_equal(out, outR), "reps variant output mismatch"
    est = (tR - t1) / (_REPS - 1)
    print(f"[timing] wall reps=1: {t1*1e6:.0f} us, reps={_REPS}: {tR*1e6:.0f} us"
          f" -> per-iter {est*1e6:.1f} us")
    return out, est


# ---------------- self-test (simulator, tiny geometry) ----------------
def _ref_numpy(x, w0, b0, w1, m):
    sx = np.array([[-1, 0, 1], [-2, 0, 2], [-1, 0, 1]], np.float32)
    sy = sx.T
    Cc, H, Wf = x.shape
    xp = np.pad(x, ((0, 0), (1, 1), (1, 1)))
    gx = np.zeros_like(x); gy = np.zeros_like(x)
    for dy in range(3):
        for dx in range(3):
            gx += sx[dy, dx] * xp[:, dy:dy + H, dx:dx + Wf]
            gy += sy[dy, dx] * xp[:, dy:dy + H, dx:dx + Wf]
    y = np.concatenate([x, gx, gy], 0).reshape(3 * Cc, -1)
    h = np.maximum(w0 @ y + b0.reshape(-1, 1), 0)
    u = (w1 @ h).reshape(Cc, H, Wf)
    return x + u * m


if __name__ == "__main__":
    from concourse.bass_interp import CoreSim
    R, WP, GR = 16, 18, 8
    Wo = WP - 2
    nc = build_nc(R=R, WP=WP, GR=GR)
    nc.finalize()
    sim = CoreSim(nc)
    rng = np.random.default_rng(0)
    x = rng.standard_normal((C, R, Wo)).astype(np.float32)
    xp_b = np.pad(x, ((0, 0), (1, 1), (1, 1))).astype(NPFX)
    ru = rng.random((R, Wo)).astype(np.float32)
    m = (ru > 0.5).astype(NPFX)
    w0 = (rng.standard_normal((HID, 3 * C)) * 0.1).astype(np.float32)
    b0 = (rng.standard_normal((HID,)) * 0.1).astype(np.float32)
    w1 = (rng.standard_normal((C, HID)) * 0.1).astype(np.float32)
    d = _pack_core(xp_b, m, R, WP, GR)
    w1t = np.zeros((HID, 32), NPFX); w1t[:, :C] = w1.T
    d.update({"w0t": np.ascontiguousarray(w0.T).astype(NPFX),
              "b0": b0.reshape(HID, 1), "w1t": w1t})
    for n, v in d.items():
        sim.tensor(n)[:] = v
    sim.simulate()
    o = np.array(sim.tensor("out")).astype(np.float32)
    n_chunk = R // GR; UTG = GR // 4
    o = o.reshape(n_chunk, 4, 32, UTG, Wo)[:, :, :C]
    got = o.transpose(2, 0, 3, 1, 4).reshape(C, R, Wo)
    exp = _ref_numpy(x, w0, b0, w1, m.astype(np.float32))
    dd = got - exp
    rel = np.linalg.norm(dd) / np.linalg.norm(exp)
    print("L2 rel err:", rel, "absmax-scale:", np.abs(dd).max() / np.abs(exp).max())
    assert rel < 2e-2, "FAIL"
    print("SIM PASS")


# revision 39
# speedup vs baseline: 1.6410x; 1.6410x over previous
"""Trainium2 Bass kernel for nn_CAModel (sobel-conv + 2-layer MLP + masked residual).

Math per pixel: y = [x, sobel_x(x), sobel_y(x)] (48 ch); h = relu(w0 @ y + b0);
u = w1 @ h; out = x + u * (rand_u > 0.5).

Sharding: data-parallel over 8 cores: (batch b, H-half) -> core b*2 + half.
Each core computes a [16, 256, 512] output slab.

Per-core design notes (cost-model-driven rewrite of the v1 kernel):
 - All inputs host-packed bf16; HWDGE DMA count minimized (each DMA holds the
   serial HWDGE ring ~630ns regardless of size).
 - Partition layout p = g*16 + c (groups of GR=16 rows x 16 channels), two
   half-images of 128 rows; 16 row-chunks of 16 rows, software-pipelined.
 - Sobel separable passes: bv (B = Dy x) on Pool, the rest on DVE (Pool
   cannot touch PSUM on HW, so DVE time is precious: T-variant avoids the
   un-2x'd scalar_tensor_tensor).  Sobel for half-image 1 and for the NEXT
   rep's half-image 0 are emitted inside pop-windows (chunks 2-6 / 9-13) so
   DVE computes them while PE crunches other chunks; reps overlap without
   any loop-carried emission (unroll>1 bodies inside one For_i iteration).
 - mm1 K=48 in row pairs -> 2-bank PSUM; one relu+bias evac per 2 rows
   (ACT mostly, DVE where free); mm2 col-tiled (tile_position) into a
   32-stacked PSUM tile.
 - Mask: host sends a replicated stacked mask slab (bf16); u*mask is one DVE
   tensor_tensor in non-window chunks, or ACT copy + Pool SBUF-multiply in
   sobel windows.
 - Residual: SWDGE accumulate-DMA adds the host-stacked x slab from HBM into
   the output staging tile; one batched bf16 store per 16-row chunk.
"""
import numpy as np
from contextlib import ExitStack

import concourse.bass as bass
import concourse.bacc as bacc
import concourse.tile as tile
from concourse import mybir

bf16 = mybir.dt.bfloat16
f32 = mybir.dt.float32
Alu = mybir.AluOpType
Act = mybir.ActivationFunctionType

C = 16          # channels
HID = 128
N_CORES = 8
NPFX = np.dtype(mybir.dt.np(bf16))  # ml_dtypes bfloat16


def build_nc(R=256, WP=514, GR=16, reps=1, unroll=1, ablate=(),
             win_dve_evac=2, psum_swap=False, nw_dve=6):
    """Per-core graph. R: out rows (2 half-images), WP: padded width, GR: rows/group."""
    W = WP - 2
    NH = 2                       # half-images
    RH = R // NH                 # rows per half
    n_g = RH // GR               # groups per half
    assert n_g * GR == RH and n_g * C <= 128
    GRH = GR + 2                 # rows incl halo
    UTG = GR // 4                # u-tiles per group (chunk)
    NP = n_g * C                 # active partitions in group layout
    WH = WP // 2 + 1             # sobel half width (overlap 2)
    n_chunk = R // GR            # output chunks (= total groups)
    SMALL = n_g < 6

    nc = bacc.Bacc()
    xg_ext = [nc.declare_dram_parameter(f"xg{h}", (NP, GRH, WP), bf16,
                                        isOutput=False) for h in range(NH)]
    xs_ext = nc.declare_dram_parameter("xs", (n_chunk, 128, UTG, W), bf16,
                                       isOutput=False)
    ms_ext = nc.declare_dram_parameter("ms", (n_chunk, 128, UTG, W), bf16,
                                       isOutput=False)
    w0_ext = nc.declare_dram_parameter("w0t", (3 * C, HID), bf16, isOutput=False)
    b0_ext = nc.declare_dram_parameter("b0", (HID, 1), f32, isOutput=False)
    w1_ext = nc.declare_dram_parameter("w1t", (HID, 32), bf16, isOutput=False)
    out_ext = nc.declare_dram_parameter("out", (n_chunk, 128, UTG, W), bf16,
                                        isOutput=True)

    with tile.TileContext(nc) as tc, ExitStack() as ctx:
        const = ctx.enter_context(tc.tile_pool(name="const", bufs=1))
        xpool = ctx.enter_context(tc.tile_pool(name="xpool", bufs=1))
        gpool = ctx.enter_context(tc.tile_pool(name="gpool", bufs=1))
        abpool = ctx.enter_context(tc.tile_pool(name="abpool", bufs=1))
        ypool = ctx.enter_context(tc.tile_pool(name="ypool", bufs=3))
        hpool = ctx.enter_context(tc.tile_pool(name="hpool", bufs=3))
        opool = ctx.enter_context(tc.tile_pool(name="opool", bufs=3))
        mpool = ctx.enter_context(tc.tile_pool(name="mpool", bufs=2))
        hpsum = ctx.enter_context(tc.tile_pool(
            name="hpsum", bufs=2 if psum_swap else 3, space="PSUM"))
        upsum = ctx.enter_context(tc.tile_pool(
            name="upsum", bufs=4 if psum_swap else 2, space="PSUM"))

        # ---- registries + prologue constants ----
        Xbs, xvs = [None, None], [None, None]
        gviews = [None, None]

        def emit_xload(h):
            Xbs[h] = xpool.tile([NP, GRH * WP], bf16, tag=f"xb{h}", name=f"xb{h}")
            xvs[h] = Xbs[h][:, :].rearrange("p (r w) -> p r w", r=GRH)
            nc.sync.dma_start(xvs[h][:, :, 0:WH], xg_ext[h][:, :, 0:WH])
            nc.sync.dma_start(xvs[h][:, :, WH:WP], xg_ext[h][:, :, WH:WP])

        W0T = const.tile([3 * C, HID], bf16, tag="w0t")
        nc.sync.dma_start(W0T[:], w0_ext[:])
        W1T = const.tile([HID, 32], bf16, tag="w1t")
        nc.sync.dma_start(W1T[:], w1_ext[:])
        B0 = const.tile([HID, 1], f32, tag="b0")
        nc.sync.dma_start(B0[:], b0_ext[:])

        def make_sobel(h):
            """Build sobel closure lists (pool_ops: bv on Pool; dve_ops: rest)."""
            GX = gpool.tile([NP, GR * W], bf16, tag=f"gx{h}", name=f"gx{h}")
            GY = gpool.tile([NP, GR * W], bf16, tag=f"gy{h}", name=f"gy{h}")
            gxv = GX[:, :].rearrange("p (r w) -> p r w", r=GR)
            gyv = GY[:, :].rearrange("p (r w) -> p r w", r=GR)
            gviews[h] = (gxv, gyv)
            xv = xvs[h]
            pool_ops, dve_ops = [], []
            for h2 in range(2):
                w0_ = h2 * (WP // 2 - 1)
                wc = h2 * (WP // 2 - 1)
                wv = slice(w0_, w0_ + WH)
                WO = WH - 2
                A = abpool.tile([NP, GR * WH], bf16, tag="a", name="a")
                B = abpool.tile([NP, GR * WH], bf16, tag="b", name="b")
                T = abpool.tile([NP, GR * WH], bf16, tag="t", name="t")
                av = A[:, :].rearrange("p (r w) -> p r w", r=GR)
                bv = B[:, :].rearrange("p (r w) -> p r w", r=GR)
                tv = T[:, :].rearrange("p (r w) -> p r w", r=GR)
                # A = x0 + x2 ; T = 2*x1 ; A += T ; GX = A>>1 - A<<1   (DVE)
                # B = x2 - x0 (Pool) ; GY = B<<1 + B>>1 ; T = 2*B ; GY += T
                pool_ops.append(
                    lambda bv=bv, xv=xv, wv=wv: nc.gpsimd.tensor_tensor(
                        bv[:, :, :], xv[:, 2:GR + 2, wv], xv[:, 0:GR, wv],
                        Alu.subtract))
                dve_ops += [
                    lambda av=av, xv=xv, wv=wv: nc.vector.tensor_tensor(
                        av[:, :, :], xv[:, 0:GR, wv], xv[:, 2:GR + 2, wv],
                        Alu.add),
                    lambda tv=tv, xv=xv, wv=wv: nc.vector.tensor_scalar(
                        tv[:, :, :], xv[:, 1:GR + 1, wv], 2.0, None, Alu.mult),
                    lambda av=av, tv=tv: nc.vector.tensor_tensor(
                        av[:, :, :], av[:, :, :], tv[:, :, :], Alu.add),
                    lambda gxv=gxv, av=av, wc=wc, WO=WO: nc.vector.tensor_tensor(
                        gxv[:, :, wc:wc + WO], av[:, :, 2:WH],
                        av[:, :, 0:WH - 2], Alu.subtract),
                    lambda gyv=gyv, bv=bv, wc=wc, WO=WO: nc.vector.tensor_tensor(
                        gyv[:, :, wc:wc + WO], bv[:, :, 0:WH - 2],
                        bv[:, :, 2:WH], Alu.add),
                    lambda tv=tv, bv=bv: nc.vector.tensor_scalar(
                        tv[:, :, :], bv[:, :, :], 2.0, None, Alu.mult),
                    lambda gyv=gyv, tv=tv, wc=wc, WO=WO: nc.vector.tensor_tensor(
                        gyv[:, :, wc:wc + WO], gyv[:, :, wc:wc + WO],
                        tv[:, :, 1:WH - 1], Alu.add),
                ]
            return pool_ops, dve_ops

        def emit_sobel_block(h):
            if 'sobel' in ablate:
                return
            pool_ops, dve_ops = make_sobel(h)
            # interleave so the pool op of each half runs early
            for op in (pool_ops[:1] + dve_ops[:7] + pool_ops[1:] + dve_ops[7:]):
                op()

        # sobel-busy chunks (UM routed ACT-copy + Pool-mult; evacs all-ACT)
        W1 = set(range(2, 7)) if not SMALL else set()
        W0_t = set(range(9, 14)) if not SMALL else set()

        def _body(first=True, last=True):
            sob_q = []
            windows = W1 | (set() if last else W0_t)
            if first:
                emit_xload(0)
                emit_sobel_block(0)
            Ys = [None] * n_chunk
            MSs = [None] * n_chunk
            OFs = [None] * n_chunk

            def pre(c):
                hh = c // n_g
                g = c % n_g
                if hh > 0 and sob_q and c < n_g + 2:
                    for op in sob_q:
                        op()
                    sob_q.clear()
                MSs[c] = mpool.tile([128, UTG * W], bf16, tag="ms", name="ms")
                nc.sync.dma_start(
                    MSs[c][:, :].rearrange("p (j w) -> p j w", j=UTG), ms_ext[c])
                Ys[c] = ypool.tile([3 * C, GR * W], bf16, tag="y", name="y")
                yv = Ys[c][:, :].rearrange("s (r w) -> s r w", r=GR)
                if 'pack' not in ablate:
                    gxv, gyv = gviews[hh]
                    nc.sync.dma_start(
                        yv[0:C, :, :],
                        xvs[hh][g * C:(g + 1) * C, 1:1 + GR, 1:WP - 1])
                    nc.sync.dma_start(yv[C:2 * C, :, :],
                                      gxv[g * C:(g + 1) * C, :, :])
                    nc.sync.dma_start(yv[2 * C:3 * C, :, :],
                                      gyv[g * C:(g + 1) * C, :, :])

            def emit_out(c, eng):
                if 'out' not in ablate:
                    ofv = OFs[c][:, :].rearrange("p (j w) -> p j w", j=UTG)
                    eng.dma_start(out_ext[c], ofv[:, :, :])

            if NH > 1:
                emit_xload(1)
                if SMALL and 'sobel' not in ablate:
                    emit_sobel_block(1)
            pre(0)
            for c in range(n_chunk):
                if c == 1 and NH > 1 and not SMALL and 'sobel' not in ablate:
                    pool_ops, dve_ops = make_sobel(1)
                    for op in pool_ops:
                        op()
                    sob_q.extend(dve_ops)
                if c == 7 and not last and not SMALL:
                    emit_xload(0)
                if c == 8 and not last and not SMALL and 'sobel' not in ablate:
                    pool_ops, dve_ops = make_sobel(0)
                    for op in pool_ops:
                        op()
                    sob_q.extend(dve_ops)
                if c == 0 and n_chunk > 1:
                    pre(1)
                yv = Ys[c][:, :].rearrange("s (r w) -> s r w", r=GR)
                msv = MSs[c][:, :].rearrange("p (j w) -> p j w", j=UTG)
                OFs[c] = opool.tile([128, UTG * W], bf16, tag="of", name="of")
                ofv = OFs[c][:, :].rearrange("p (j w) -> p j w", j=UTG)
                in_win = c in windows
                for j in range(UTG):
                    hsb2 = []
                    for half in range(2):
                        h_ps = hpsum.tile([HID, 2 * W], f32, tag="h2", name="h2")
                        if 'mm1' not in ablate:
                            for kk in range(2):
                                r = 4 * j + 2 * half + kk
                                nc.tensor.matmul(h_ps[:, kk * W:(kk + 1) * W],
                                                 W0T[:], yv[:, r, :],
                                                 start=True, stop=True)
                        h_sb = hpool.tile([HID, 2 * W], bf16, tag="h2s",
                                          name="h2s")
                        hsb2.append(h_sb)
                        if 'evac' not in ablate:
                            u_i = 2 * j + half
                            dve_evac = (u_i % 2 == 0 or
                                        (nw_dve >= 5 and u_i == 1) or
                                        (nw_dve >= 6 and u_i == 3) or
                                        (nw_dve >= 7 and u_i == 5)) \
                                if not in_win \
                                else (win_dve_evac == 4 and u_i % 2 == 1
                                      or win_dve_evac == 2 and u_i % 4 == 1)
                            if dve_evac:
                                nc.vector.tensor_scalar(h_sb[:], h_ps[:],
                                                        B0[:], 0.0,
                                                        Alu.add, Alu.max)
                            else:
                                nc.scalar.activation(h_sb[:], h_ps[:],
                                                     Act.Relu, bias=B0[:])

                    u_ps = upsum.tile([128, W], f32, tag="u", name="u")
                    if 'mm2' not in ablate:
                        for k in range(4):
                            nc.tensor.matmul(
                                u_ps[32 * k:32 * k + 32, :], W1T[:],
                                hsb2[k // 2][:, (k % 2) * W:(k % 2 + 1) * W],
                                start=True, stop=True,
                                tile_position=(0, 32 * k))
                    if 'um' not in ablate:
                        if in_win:
                            nc.scalar.activation(ofv[:, j, :], u_ps[:],
                                                 Act.Copy)
                            nc.gpsimd.tensor_tensor(ofv[:, j, :],
                                                    ofv[:, j, :],
                                                    msv[:, j, :], Alu.mult)
                        else:
                            nc.vector.tensor_tensor(ofv[:, j, :], u_ps[:],
                                                    msv[:, j, :], Alu.mult)
                    # staggered non-compute work
                    if j == 0 and c >= 2:
                        emit_out(c - 2, nc.sync)
                    if j == 1 and c + 2 < n_chunk:
                        pre(c + 2)
                    if sob_q and (in_win or c in (7, 8)):
                        take = 1 if c < n_chunk - 1 else len(sob_q)
                        for op in sob_q[:take]:
                            op()
                        del sob_q[:take]
                if 'accum' not in ablate:
                    nc.gpsimd.dma_start(ofv[:, :, :], xs_ext[c],
                                        accum_op=Alu.add)
            for op in sob_q:
                op()
            for c in range(max(0, n_chunk - 2), n_chunk):
                emit_out(c, nc.scalar)

        assert reps % unroll == 0
        if reps > 1:
            with tc.For_i(0, reps // unroll, 1):
                for r in range(unroll):
                    _body(first=(r == 0), last=(r == unroll - 1))
        else:
            _body()
    return nc


_CACHE = {}


def _get_nc():
    if "nc" not in _CACHE:
        nc = build_nc()
        nc.finalize()
        _CACHE["nc"] = nc
    return _CACHE["nc"]


def _pack_core(xp_b, m_b, R, WP, GR):
    """Per-core input pack. xp_b: [C, R+2, WP] padded bf16 slab (rows incl
    halo), m_b: [R, W] bf16 thresholded mask. Returns dict of arrays."""
    W = WP - 2
    NH = 2
    RH = R // NH
    n_g = RH // GR
    NP = n_g * C
    GRH = GR + 2
    UTG = GR // 4
    n_chunk = R // GR

    d = {}
    for h in range(NH):
        xg = np.empty((NP, GRH, WP), NPFX)
        for g in range(n_g):
            r0 = h * RH + g * GR
            xg[g * C:(g + 1) * C] = xp_b[:, r0:r0 + GRH, :]
        d[f"xg{h}"] = xg
    # xs[chunk, 32k+c, j, w] = x[c, GR*chunk + 4j + k, w] (interior), c>=16 -> 0
    xs = np.zeros((n_chunk, 128, UTG, W), NPFX)
    xint = xp_b[:, 1:R + 1, 1:WP - 1]                  # [C, R, W]
    for k in range(4):
        xs[:, 32 * k:32 * k + C] = (
            xint[:, k::4, :].reshape(C, n_chunk, UTG, W).transpose(1, 0, 2, 3))
    d["xs"] = xs
    # ms[chunk, 32k+c, j, w] = m[GR*chunk + 4j + k, w]  (all c)
    ms = np.empty((n_chunk, 128, UTG, W), NPFX)
    mm = m_b.reshape(n_chunk, UTG, 4, W)
    for k in range(4):
        ms[:, 32 * k:32 * k + 32] = mm[:, :, k][:, None, :, :]
    d["ms"] = ms
    return d


def _shard_inputs(x, w0, b0, w1, rand_u, R=256, WP=514, GR=16):
    B, _, H, Wf = x.shape
    xp = np.pad(x, ((0, 0), (0, 0), (1, 1), (1, 1))).astype(NPFX)
    m = (rand_u[:, 0] > 0.5).astype(NPFX)              # [B, H, W]
    w0t = np.ascontiguousarray(w0.T).astype(NPFX)      # [48, 128]
    b0c = np.ascontiguousarray(b0, np.float32).reshape(HID, 1)
    w1t = np.zeros((HID, 32), NPFX)
    w1t[:, :C] = w1.T
    in_maps = []
    for core in range(N_CORES):
        b, hh = divmod(core, 2)
        rowbase = hh * R
        xp_b = xp[b, :, rowbase:rowbase + R + 2, :]    # [C, R+2, WP]
        m_b = m[b, rowbase:rowbase + R, :]             # [R, W]
        dd = _pack_core(xp_b, m_b, R, WP, GR)
        dd.update({"w0t": w0t, "b0": b0c, "w1t": w1t})
        in_maps.append(dd)
    return in_maps


def _assemble(results, B, H, Wf, R=256, GR=16):
    out = np.empty((B, C, H, Wf), np.float32)
    n_chunk = R // GR
    UTG = GR // 4
    for core, res in enumerate(results):
        b, hh = divmod(core, 2)
        o = np.asarray(res["out"]).astype(np.float32)  # [n_chunk, 128, UTG, W]
        o = o.reshape(n_chunk, 4, 32, UTG, Wf)[:, :, :C]   # [nc, 4(k), 16, UTG, W]
        o = o.transpose(2, 0, 3, 1, 4).reshape(C, R, Wf)
        out[b, :, hh * R:(hh + 1) * R, :] = o
    return out


def kernel(x, w0, b0, w1, rand_u, _trace=False):
    from concourse.bass_utils import run_bass_kernel_spmd
    nc = _get_nc()
    in_maps = _shard_inputs(x, w0, b0, w1, rand_u)
    res = run_bass_kernel_spmd(nc, in_maps, core_ids=list(range(N_CORES)))
    out = _assemble(res.results, x.shape[0], x.shape[2], x.shape[3])
    if _trace:
        return out, res
    return out


def _run_timed(nc, in_maps, iters):
    import time
    import jax
    from concourse import mybir
    from jax.sharding import Mesh, PartitionSpec
    from jax.experimental.shard_map import shard_map
    from concourse import bass2jax
    from concourse.bass2jax import _bass_exec_p

    bass2jax.install_neuronx_cc_hook()

    pname = nc.partition_id_tensor.name if nc.partition_id_tensor else None
    in_names, out_names, out_avals, zero_outs = [], [], [], []
    for alloc in nc.m.functions[0].allocations:
        if not isinstance(alloc, mybir.MemoryLocationSet):
            continue
        name = alloc.memorylocations[0].name
        if alloc.kind == "ExternalInput":
            if name != pname:
                in_names.append(name)
        elif alloc.kind == "ExternalOutput":
            out_names.append(name)
            shape = tuple(alloc.tensor_shape)
            np_dt = mybir.dt.np(alloc.dtype)
            out_avals.append(jax.core.ShapedArray(shape, np_dt))
            zero_outs.append(np.zeros(shape, np_dt))
    n_params = len(in_names)
    all_in = in_names + out_names
    if pname is not None:
        all_in = all_in + [pname]

    def _body(*args):
        operands = list(args)
        if pname is not None:
            operands.append(bass2jax.partition_id_tensor())
        outs = _bass_exec_p.bind(
            *operands, out_avals=tuple(out_avals), in_names=tuple(all_in),
            out_names=tuple(out_names), lowering_input_output_aliases=(),
            sim_require_finite=True, sim_require_nnan=True, nc=nc)
        return tuple(outs)

    devices = jax.devices()[:N_CORES]
    mesh = Mesh(np.asarray(devices), ("core",))
    specs = (PartitionSpec("core"),)
    fn = jax.jit(shard_map(_body, mesh=mesh,
                           in_specs=specs * (n_params + len(out_names)),
                           out_specs=specs * len(out_names), check_rep=False),
                 keep_unused=True)
    concat_in = [np.concatenate([np.asarray(in_maps[c][n]) for c in range(N_CORES)], axis=0)
                 for n in in_names]
    concat_zeros = [np.zeros((N_CORES * z.shape[0], *z.shape[1:]), z.dtype)
                    for z in zero_outs]
    dev_in = [jax.device_put(a) for a in concat_in + concat_zeros]

    outs = fn(*dev_in)
    jax.block_until_ready(outs)
    best = float("inf")
    for _ in range(iters):
        t0 = time.perf_counter()
        outs = fn(*dev_in)
        jax.block_until_ready(outs)
        best = min(best, time.perf_counter() - t0)

    res = [{n: np.asarray(outs[i]).reshape(N_CORES, *out_avals[i].shape)[c]
            for i, n in enumerate(out_names)} for c in range(N_CORES)]
    return res, best


_REPS = 256
_UNROLL = 4


def kernel_timed(x, w0, b0, w1, rand_u, iters=8):
    """Returns (out, est_exec_seconds): marginal per-iteration silicon time
    measured as (wall(reps=N) - wall(reps=1)) / (N-1) on device-resident inputs."""
    in_maps = _shard_inputs(x, w0, b0, w1, rand_u)
    nc1 = _get_nc()
    res, t1 = _run_timed(nc1, in_maps, iters)
    out = _assemble(res, x.shape[0], x.shape[2], x.shape[3])
    if "ncR" not in _CACHE:
        ncR = build_nc(reps=_REPS, unroll=_UNROLL)
        ncR.finalize()
        _CACHE["ncR"] = ncR
    resR, tR = _run_timed(_CACHE["ncR"], in_maps, iters)
    outR = _assemble(resR, x.shape[0], x.shape[2], x.shape[3])
    assert np.array_equal(out, outR), "reps variant output mismatch"
    est = (tR - t1) / (_REPS - 1)
    print(f"[timing] wall reps=1: {t1*1e6:.0f} us, reps={_REPS}: {tR*1e6:.0f} us"
          f" -> per-iter {est*1e6:.1f} us")
    return out, est


# ---------------- self-test (simulator, tiny geometry) ----------------
def _ref_numpy(x, w0, b0, w1, m):
    sx = np.array([[-1, 0, 1], [-2, 0, 2], [-1, 0, 1]], np.float32)
    sy = sx.T
    Cc, H, Wf = x.shape
    xp = np.pad(x, ((0, 0), (1, 1), (1, 1)))
    gx = np.zeros_like(x); gy = np.zeros_like(x)
    for dy in range(3):
        for dx in range(3):
            gx += sx[dy, dx] * xp[:, dy:dy + H, dx:dx + Wf]
            gy += sy[dy, dx] * xp[:, dy:dy + H, dx:dx + Wf]
    y = np.concatenate([x, gx, gy], 0).reshape(3 * Cc, -1)
    h = np.maximum(w0 @ y + b0.reshape(-1, 1), 0)
    u = (w1 @ h).reshape(Cc, H, Wf)
    return x + u * m


if __name__ == "__main__":
    from concourse.bass_interp import CoreSim
    R, WP, GR = 16, 18, 8
    Wo = WP - 2
    nc = build_nc(R=R, WP=WP, GR=GR)
    nc.finalize()
    sim = CoreSim(nc)
    rng = np.random.default_rng(0)
    x = rng.standard_normal((C, R, Wo)).astype(np.float32)
    xp_b = np.pad(x, ((0, 0), (1, 1), (1, 1))).astype(NPFX)
    ru = rng.random((R, Wo)).astype(np.float32)
    m = (ru > 0.5).astype(NPFX)
    w0 = (rng.standard_normal((HID, 3 * C)) * 0.1).astype(np.float32)
    b0 = (rng.standard_normal((HID,)) * 0.1).astype(np.float32)
    w1 = (rng.standard_normal((C, HID)) * 0.1).astype(np.float32)
    d = _pack_core(xp_b, m, R, WP, GR)
    w1t = np.zeros((HID, 32), NPFX); w1t[:, :C] = w1.T
    d.update({"w0t": np.ascontiguousarray(w0.T).astype(NPFX),
              "b0": b0.reshape(HID, 1), "w1t": w1t})
    for n, v in d.items():
        sim.tensor(n)[:] = v
    sim.simulate()
    o = np.array(sim.tensor("out")).astype(np.float32)
    n_chunk = R // GR; UTG = GR // 4
    o = o.reshape(n_chunk, 4, 32, UTG, Wo)[:, :, :C]
    got = o.transpose(2, 0, 3, 1, 4).reshape(C, R, Wo)
    exp = _ref_numpy(x, w0, b0, w1, m.astype(np.float32))
    dd = got - exp
    rel = np.linalg.norm(dd) / np.linalg.norm(exp)
    print("L2 rel err:", rel, "absmax-scale:", np.abs(dd).max() / np.abs(exp).max())
    assert rel < 2e-2, "FAIL"
    print("SIM PASS")
